# revision 9
# baseline (speedup 1.0000x reference)
"""Trainium2 Bass kernel for nn_ConvblockNofrills (dense_cnn).

Reference computation (per batch b, output position l, channel d):
    gate[b,l,d,k] = tanh( sum_c x[b, l+K-1, c] * weights[d, c, k] )
    out[b,l,d]    = sum_k x[b, l+k, d] * gate[b,l,d,k]
with B=8, T=4096, C=D=512, K=7, L=T-K+1=4090.

Strategy: data-parallel across the 8 NeuronCores (one batch each).
Per core everything runs in transposed (channel, position) layout:
  - gates via bf16 matmul on TensorE (fp32 PSUM accumulation)
  - tanh on ScalarE (fp32-accurate spline), output bf16 to SBUF
  - 7-tap multiply/accumulate on VectorE in bf16
Loop order (lq, k, dc, cc, lt) with DMA loads issued in consumption
order, so the tensor engine starts after ~1MB has landed and never
starves: the k=0 sweep over all dc needs only w[k=0] plus the first
position blocks of x, and each later k-sweep adds one 0.5MB w slice.
Host side transposes/casts inputs (part of sharding) and transposes the
(C, L) bf16 per-core result back to the (B, L, C) fp32 output.
"""

import numpy as np
import ml_dtypes

import sys
for _p in ("/opt/trn_rl_repo", "/root/.axon_site/_ro/trn_rl_repo"):
    if _p not in sys.path:
        sys.path.append(_p)

B, T, C, K = 8, 4096, 512, 7
L = T - K + 1  # 4090
NCORES = 8
P = 128           # partitions
DC = C // P       # 4 channel chunks
NL = 512          # l-tile (one PSUM bank of fp32)
NLT = (L + NL - 1) // NL  # 8 l-tiles, last ragged (506)
QUAD = 4          # l-tiles per group
NQ = NLT // QUAD  # 2 groups
XBLK = 512        # x load chunk (columns)

_cache = {}


def _build():
    import concourse.bass as bass  # noqa: F401
    import concourse.mybir as mybir
    import concourse.tile as tile
    from concourse import bacc

    bf16 = mybir.dt.bfloat16
    f32 = mybir.dt.float32
    Tanh = mybir.ActivationFunctionType.Tanh

    nc = bacc.Bacc("TRN2", target_bir_lowering=False, debug=False,
                   num_devices=NCORES)

    xT_d = nc.dram_tensor("xT", [C, T], bf16, kind="ExternalInput")
    wT_d = nc.dram_tensor("wT", [K, C, C], bf16, kind="ExternalInput")
    outT_d = nc.dram_tensor("outT", [C, L], bf16, kind="ExternalOutput")

    with tile.TileContext(nc) as tc:
        with (
            tc.tile_pool(name="wpool", bufs=1) as wpool,
            tc.tile_pool(name="xpool", bufs=1) as xpool,
            tc.tile_pool(name="gpool", bufs=6) as gpool,
            tc.tile_pool(name="apool", bufs=8) as apool,
            tc.tile_pool(name="ppool", bufs=3) as ppool,
            tc.tile_pool(name="psum", bufs=8, space="PSUM") as psum_pool,
        ):
            # w_sb[cc][c_in, k, d] = weights[d, cc*128+c_in, k]
            w_sb = [wpool.tile([P, K, C], bf16, name=f"w_{cc}")
                    for cc in range(DC)]
            xT_sb = [xpool.tile([P, T], bf16, name=f"xt_{cc}")
                     for cc in range(DC)]

            def load_w(k):
                for cc in range(DC):
                    nc.sync.dma_start(w_sb[cc][:, k, :],
                                      wT_d.ap()[k, cc * P:(cc + 1) * P, :])

            def load_x(blk):
                c0 = blk * XBLK
                for cc in range(DC):
                    nc.sync.dma_start(
                        xT_sb[cc][:, c0:c0 + XBLK],
                        xT_d.ap()[cc * P:(cc + 1) * P, c0:c0 + XBLK])

            # DMA issue order == consumption order. (Finer splits were
            # tried and regress: more dma_starts serialize on the sync
            # dispatcher and smaller descriptors cut DMA efficiency.)
            for cc in range(DC):
                nc.sync.dma_start(w_sb[cc][:, 0, :],
                                  wT_d.ap()[0, cc * P:(cc + 1) * P, :])
                for blk in (0, 1):
                    c0 = blk * XBLK
                    nc.sync.dma_start(
                        xT_sb[cc][:, c0:c0 + XBLK],
                        xT_d.ap()[cc * P:(cc + 1) * P, c0:c0 + XBLK])
            for blk in range(2, 5):    # cols 0..2559 cover quad 0 (+halo)
                load_x(blk)
            for k in range(1, K):
                load_w(k)
            for blk in range(5, T // XBLK):
                load_x(blk)

            # PE clock ramps (~0.9 -> 2.4 GHz) over the first ~10us of
            # activity, and re-throttles after any >1us idle gap. Run
            # dummy matmuls on a scratch tile sized so the PE exits
            # warm-up right when the first k-sweep's ~3MB of operands
            # has landed (~18us) and can stream gap-free at full clock.
            # (The psum result is never read.)
            NWARM = 34
            warm = wpool.tile([P, NL], bf16, name="warm")
            nc.gpsimd.memset(warm[:], 1.0)
            warm_ps = psum_pool.tile([P, NL], f32, tag="ps", name="warm_ps")
            for i in range(NWARM):
                nc.tensor.matmul(warm_ps, warm[:, :P], warm,
                                 start=True, stop=(i == NWARM - 1))

            for lq in range(NQ):
                q0 = lq * QUAD * NL               # first output col
                qn = min(QUAD * NL, L - q0)       # 2048 / 2042
                acc = [None] * DC
                for k in range(K):
                    for dc in range(DC):
                        ps = [psum_pool.tile([P, NL], f32, tag="ps",
                                             name=f"ps_{lq}_{k}_{dc}_{i}")
                              for i in range(QUAD)]
                        for cc in range(DC):
                            lhsT = w_sb[cc][:, k, dc * P:(dc + 1) * P]
                            for i in range(QUAD):
                                l0 = q0 + i * NL
                                nl = min(NL, L - l0)
                                nc.tensor.matmul(
                                    ps[i][:, :nl],
                                    lhsT,
                                    xT_sb[cc][:, l0 + K - 1: l0 + K - 1 + nl],
                                    start=(cc == 0),
                                    stop=(cc == DC - 1),
                                )
                        g = gpool.tile([P, QUAD * NL], bf16, tag="g",
                                       name=f"g_{lq}_{k}_{dc}")
                        last_unit = (lq == NQ - 1 and k == K - 1
                                     and dc == DC - 1)
                        if last_unit:
                            # Pipeline the final unit per l-tile so the
                            # epilogue after the last matmul is short.
                            nxt = apool.tile([P, QUAD * NL], bf16, tag="acc",
                                             name=f"accn_{lq}_{k}_{dc}")
                            for i in range(QUAD):
                                l0 = q0 + i * NL
                                nl = min(NL, L - l0)
                                o = i * NL
                                gs = g[:, o:o + nl]
                                nc.scalar.activation(gs, ps[i][:, :nl], Tanh)
                                prod = ppool.tile([P, QUAD * NL], bf16,
                                                  tag="prod",
                                                  name=f"prodL_{i}")
                                nc.vector.tensor_mul(
                                    prod[:, o:o + nl], gs,
                                    xT_sb[dc][:, l0 + k:l0 + k + nl])
                                nc.vector.tensor_add(
                                    nxt[:, o:o + nl],
                                    acc[dc][:, o:o + nl], prod[:, o:o + nl])
                                nc.sync.dma_start(
                                    outT_d.ap()[dc * P:(dc + 1) * P,
                                                l0:l0 + nl],
                                    nxt[:, o:o + nl])
                            acc[dc] = None
                            continue
                        for i in range(QUAD):
                            l0 = q0 + i * NL
                            nl = min(NL, L - l0)
                            nc.scalar.activation(
                                g[:, i * NL:i * NL + nl], ps[i][:, :nl], Tanh)
                        xu = xT_sb[dc][:, q0 + k:q0 + k + qn]
                        if acc[dc] is None:
                            a0 = apool.tile([P, QUAD * NL], bf16, tag="acc",
                                            name=f"acc_{lq}_{k}_{dc}")
                            nc.vector.tensor_mul(a0[:, :qn], g[:, :qn], xu)
                            acc[dc] = a0
                        else:
                            prod = ppool.tile([P, QUAD * NL], bf16,
                                              tag="prod",
                                              name=f"prod_{lq}_{k}_{dc}")
                            nc.vector.tensor_mul(prod[:, :qn], g[:, :qn], xu)
                            nxt = apool.tile([P, QUAD * NL], bf16, tag="acc",
                                             name=f"accn_{lq}_{k}_{dc}")
                            nc.vector.tensor_add(nxt[:, :qn], acc[dc][:, :qn],
                                                 prod[:, :qn])
                            acc[dc] = nxt
                for dc in range(DC):
                    if acc[dc] is not None:
                        nc.sync.dma_start(
                            outT_d.ap()[dc * P:(dc + 1) * P, q0:q0 + qn],
                            acc[dc][:, :qn])

    nc.compile()
    return nc


def _prep_inputs(x, weights):
    bf = ml_dtypes.bfloat16
    wT = np.transpose(weights, (2, 1, 0)).astype(bf)  # (K, C, D)
    wT = np.ascontiguousarray(wT)
    in_maps = []
    for b in range(B):
        xT = x[b].T.astype(bf)  # (C, T) contiguous
        in_maps.append({"xT": xT, "wT": wT})
    return in_maps


def kernel(x, weights):
    x = np.asarray(x, dtype=np.float32)
    weights = np.asarray(weights, dtype=np.float32)
    assert x.shape == (B, T, C) and weights.shape == (C, C, K)

    from concourse.bass_utils import run_bass_kernel_spmd

    if "nc" not in _cache:
        _cache["nc"] = _build()
    nc = _cache["nc"]

    in_maps = _prep_inputs(x, weights)
    res = run_bass_kernel_spmd(nc, in_maps, list(range(NCORES)))

    out = np.empty((B, L, C), dtype=np.float32)
    for b in range(B):
        out[b] = res.results[b]["outT"].astype(np.float32).T
    return out


if __name__ == "__main__":
    rng = np.random.default_rng(0)
    x = rng.standard_normal((B, T, C), dtype=np.float32)
    w = (rng.standard_normal((C, C, K), dtype=np.float32)
         / np.sqrt(np.float32(C * K)))
    out = kernel(x, w)
    print("out", out.shape, out.dtype, float(np.abs(out).max()))


# revision 11
# speedup vs baseline: 1.0128x; 1.0128x over previous
"""Trainium2 Bass kernel for nn_ConvblockNofrills (dense_cnn).

Reference computation (per batch b, output position l, channel d):
    gate[b,l,d,k] = tanh( sum_c x[b, l+K-1, c] * weights[d, c, k] )
    out[b,l,d]    = sum_k x[b, l+k, d] * gate[b,l,d,k]
with B=8, T=4096, C=D=512, K=7, L=T-K+1=4090.

Strategy: data-parallel across the 8 NeuronCores (one batch each).
Per core everything runs in transposed (channel, position) layout:
  - gates via bf16 matmul on TensorE (fp32 PSUM accumulation)
  - tanh on ScalarE (fp32-accurate spline), output bf16 to SBUF
  - 7-tap multiply/accumulate on VectorE in bf16
Loop order (lq, k, dc, cc, lt) with DMA loads issued in consumption
order, so the tensor engine starts after ~1MB has landed and never
starves: the k=0 sweep over all dc needs only w[k=0] plus the first
position blocks of x, and each later k-sweep adds one 0.5MB w slice.
Host side transposes/casts inputs (part of sharding) and transposes the
(C, L) bf16 per-core result back to the (B, L, C) fp32 output.
"""

import numpy as np
import ml_dtypes

import sys
for _p in ("/opt/trn_rl_repo", "/root/.axon_site/_ro/trn_rl_repo"):
    if _p not in sys.path:
        sys.path.append(_p)

B, T, C, K = 8, 4096, 512, 7
L = T - K + 1  # 4090
NCORES = 8
P = 128           # partitions
DC = C // P       # 4 channel chunks
NL = 512          # l-tile (one PSUM bank of fp32)
NLT = (L + NL - 1) // NL  # 8 l-tiles, last ragged (506)
QUAD = 4          # l-tiles per group
NQ = NLT // QUAD  # 2 groups
XBLK = 512        # x load chunk (columns)

_cache = {}


def _build():
    import concourse.bass as bass  # noqa: F401
    import concourse.mybir as mybir
    import concourse.tile as tile
    from concourse import bacc

    bf16 = mybir.dt.bfloat16
    f32 = mybir.dt.float32
    Tanh = mybir.ActivationFunctionType.Tanh

    nc = bacc.Bacc("TRN2", target_bir_lowering=False, debug=False,
                   num_devices=NCORES)

    xT_d = nc.dram_tensor("xT", [C, T], bf16, kind="ExternalInput")
    wT_d = nc.dram_tensor("wT", [K, C, C], bf16, kind="ExternalInput")
    outT_d = nc.dram_tensor("outT", [C, L], bf16, kind="ExternalOutput")

    with tile.TileContext(nc) as tc:
        with (
            tc.tile_pool(name="wpool", bufs=1) as wpool,
            tc.tile_pool(name="xpool", bufs=1) as xpool,
            tc.tile_pool(name="gpool", bufs=6) as gpool,
            tc.tile_pool(name="apool", bufs=8) as apool,
            tc.tile_pool(name="ppool", bufs=3) as ppool,
            tc.tile_pool(name="psum", bufs=8, space="PSUM") as psum_pool,
        ):
            # w_sb[cc][c_in, k, d] = weights[d, cc*128+c_in, k]
            w_sb = [wpool.tile([P, K, C], bf16, name=f"w_{cc}")
                    for cc in range(DC)]
            xT_sb = [xpool.tile([P, T], bf16, name=f"xt_{cc}")
                     for cc in range(DC)]

            def load_w(k):
                for cc in range(DC):
                    nc.sync.dma_start(w_sb[cc][:, k, :],
                                      wT_d.ap()[k, cc * P:(cc + 1) * P, :])

            def load_x(blk):
                c0 = blk * XBLK
                for cc in range(DC):
                    nc.sync.dma_start(
                        xT_sb[cc][:, c0:c0 + XBLK],
                        xT_d.ap()[cc * P:(cc + 1) * P, c0:c0 + XBLK])

            # DMA issue order == consumption order. (Finer splits were
            # tried and regress: more dma_starts serialize on the sync
            # dispatcher and smaller descriptors cut DMA efficiency.)
            for cc in range(DC):
                nc.sync.dma_start(w_sb[cc][:, 0, :],
                                  wT_d.ap()[0, cc * P:(cc + 1) * P, :])
                for blk in (0, 1):
                    c0 = blk * XBLK
                    nc.sync.dma_start(
                        xT_sb[cc][:, c0:c0 + XBLK],
                        xT_d.ap()[cc * P:(cc + 1) * P, c0:c0 + XBLK])
            for blk in range(2, 5):    # cols 0..2559 cover quad 0 (+halo)
                load_x(blk)
            for k in range(1, K):
                load_w(k)
            for blk in range(5, T // XBLK):
                load_x(blk)

            # PE clock ramps (~0.9 -> 2.4 GHz) over the first ~10us of
            # activity, and re-throttles after any >1us idle gap. Run
            # dummy matmuls on a scratch tile sized so the PE exits
            # warm-up right when the first k-sweep's ~3MB of operands
            # has landed (~18us) and can stream gap-free at full clock.
            # (The psum result is never read.)
            NWARM = 30
            warm = wpool.tile([P, NL], bf16, name="warm")
            nc.gpsimd.memset(warm[:], 1.0)
            warm_ps = psum_pool.tile([P, NL], f32, tag="ps", name="warm_ps")
            for i in range(NWARM):
                nc.tensor.matmul(warm_ps, warm[:, :P], warm,
                                 start=True, stop=(i == NWARM - 1))

            for lq in range(NQ):
                q0 = lq * QUAD * NL               # first output col
                qn = min(QUAD * NL, L - q0)       # 2048 / 2042
                acc = [None] * DC
                if lq == 0:
                    # Split the first k-sweep into two half-quad passes:
                    # the first pass only needs x cols 0..1535 (~2MB with
                    # w[k=0]), so matmuls stream gap-free while the rest
                    # of the quad's operands are still in flight.
                    g0 = [gpool.tile([P, QUAD * NL], bf16, tag="g",
                                     name=f"g0_{dc}") for dc in range(DC)]
                    for half in range(2):
                        for dc in range(DC):
                            ps = [psum_pool.tile([P, NL], f32, tag="ps",
                                                 name=f"ps0_{half}_{dc}_{i}")
                                  for i in range(2)]
                            for cc in range(DC):
                                lhsT = w_sb[cc][:, 0, dc * P:(dc + 1) * P]
                                for i in range(2):
                                    lt = half * 2 + i
                                    l0 = lt * NL
                                    nc.tensor.matmul(
                                        ps[i],
                                        lhsT,
                                        xT_sb[cc][:, l0 + K - 1:
                                                  l0 + K - 1 + NL],
                                        start=(cc == 0),
                                        stop=(cc == DC - 1),
                                    )
                            for i in range(2):
                                lt = half * 2 + i
                                nc.scalar.activation(
                                    g0[dc][:, lt * NL:(lt + 1) * NL],
                                    ps[i], Tanh)
                    for dc in range(DC):
                        a0 = apool.tile([P, QUAD * NL], bf16, tag="acc",
                                        name=f"acc0_{dc}")
                        nc.vector.tensor_mul(a0[:, :qn], g0[dc][:, :qn],
                                             xT_sb[dc][:, q0:q0 + qn])
                        acc[dc] = a0
                for k in range(K):
                    if lq == 0 and k == 0:
                        continue
                    for dc in range(DC):
                        ps = [psum_pool.tile([P, NL], f32, tag="ps",
                                             name=f"ps_{lq}_{k}_{dc}_{i}")
                              for i in range(QUAD)]
                        for cc in range(DC):
                            lhsT = w_sb[cc][:, k, dc * P:(dc + 1) * P]
                            for i in range(QUAD):
                                l0 = q0 + i * NL
                                nl = min(NL, L - l0)
                                nc.tensor.matmul(
                                    ps[i][:, :nl],
                                    lhsT,
                                    xT_sb[cc][:, l0 + K - 1: l0 + K - 1 + nl],
                                    start=(cc == 0),
                                    stop=(cc == DC - 1),
                                )
                        g = gpool.tile([P, QUAD * NL], bf16, tag="g",
                                       name=f"g_{lq}_{k}_{dc}")
                        last_unit = (lq == NQ - 1 and k == K - 1
                                     and dc == DC - 1)
                        if last_unit:
                            # Pipeline the final unit per l-tile so the
                            # epilogue after the last matmul is short.
                            nxt = apool.tile([P, QUAD * NL], bf16, tag="acc",
                                             name=f"accn_{lq}_{k}_{dc}")
                            for i in range(QUAD):
                                l0 = q0 + i * NL
                                nl = min(NL, L - l0)
                                o = i * NL
                                gs = g[:, o:o + nl]
                                nc.scalar.activation(gs, ps[i][:, :nl], Tanh)
                                prod = ppool.tile([P, QUAD * NL], bf16,
                                                  tag="prod",
                                                  name=f"prodL_{i}")
                                nc.vector.tensor_mul(
                                    prod[:, o:o + nl], gs,
                                    xT_sb[dc][:, l0 + k:l0 + k + nl])
                                nc.vector.tensor_add(
                                    nxt[:, o:o + nl],
                                    acc[dc][:, o:o + nl], prod[:, o:o + nl])
                                nc.sync.dma_start(
                                    outT_d.ap()[dc * P:(dc + 1) * P,
                                                l0:l0 + nl],
                                    nxt[:, o:o + nl])
                            acc[dc] = None
                            continue
                        for i in range(QUAD):
                            l0 = q0 + i * NL
                            nl = min(NL, L - l0)
                            nc.scalar.activation(
                                g[:, i * NL:i * NL + nl], ps[i][:, :nl], Tanh)
                        xu = xT_sb[dc][:, q0 + k:q0 + k + qn]
                        if acc[dc] is None:
                            a0 = apool.tile([P, QUAD * NL], bf16, tag="acc",
                                            name=f"acc_{lq}_{k}_{dc}")
                            nc.vector.tensor_mul(a0[:, :qn], g[:, :qn], xu)
                            acc[dc] = a0
                        else:
                            prod = ppool.tile([P, QUAD * NL], bf16,
                                              tag="prod",
                                              name=f"prod_{lq}_{k}_{dc}")
                            nc.vector.tensor_mul(prod[:, :qn], g[:, :qn], xu)
                            nxt = apool.tile([P, QUAD * NL], bf16, tag="acc",
                                             name=f"accn_{lq}_{k}_{dc}")
                            nc.vector.tensor_add(nxt[:, :qn], acc[dc][:, :qn],
                                                 prod[:, :qn])
                            acc[dc] = nxt
                for dc in range(DC):
                    if acc[dc] is not None:
                        nc.sync.dma_start(
                            outT_d.ap()[dc * P:(dc + 1) * P, q0:q0 + qn],
                            acc[dc][:, :qn])

    nc.compile()
    return nc


def _prep_inputs(x, weights):
    bf = ml_dtypes.bfloat16
    wT = np.transpose(weights, (2, 1, 0)).astype(bf)  # (K, C, D)
    wT = np.ascontiguousarray(wT)
    in_maps = []
    for b in range(B):
        xT = x[b].T.astype(bf)  # (C, T) contiguous
        in_maps.append({"xT": xT, "wT": wT})
    return in_maps


def kernel(x, weights):
    x = np.asarray(x, dtype=np.float32)
    weights = np.asarray(weights, dtype=np.float32)
    assert x.shape == (B, T, C) and weights.shape == (C, C, K)

    from concourse.bass_utils import run_bass_kernel_spmd

    if "nc" not in _cache:
        _cache["nc"] = _build()
    nc = _cache["nc"]

    in_maps = _prep_inputs(x, weights)
    res = run_bass_kernel_spmd(nc, in_maps, list(range(NCORES)))

    out = np.empty((B, L, C), dtype=np.float32)
    for b in range(B):
        out[b] = res.results[b]["outT"].astype(np.float32).T
    return out


if __name__ == "__main__":
    rng = np.random.default_rng(0)
    x = rng.standard_normal((B, T, C), dtype=np.float32)
    w = (rng.standard_normal((C, C, K), dtype=np.float32)
         / np.sqrt(np.float32(C * K)))
    out = kernel(x, w)
    print("out", out.shape, out.dtype, float(np.abs(out).max()))


# revision 12
# speedup vs baseline: 1.0144x; 1.0016x over previous
"""Trainium2 Bass kernel for nn_ConvblockNofrills (dense_cnn).

Reference computation (per batch b, output position l, channel d):
    gate[b,l,d,k] = tanh( sum_c x[b, l+K-1, c] * weights[d, c, k] )
    out[b,l,d]    = sum_k x[b, l+k, d] * gate[b,l,d,k]
with B=8, T=4096, C=D=512, K=7, L=T-K+1=4090.

Strategy: data-parallel across the 8 NeuronCores (one batch each).
Per core everything runs in transposed (channel, position) layout:
  - gates via bf16 matmul on TensorE (fp32 PSUM accumulation)
  - tanh on ScalarE (fp32-accurate spline), output bf16 to SBUF
  - 7-tap multiply/accumulate on VectorE in bf16
Loop order (lq, k, dc, cc, lt) with DMA loads issued in consumption
order, so the tensor engine starts after ~1MB has landed and never
starves: the k=0 sweep over all dc needs only w[k=0] plus the first
position blocks of x, and each later k-sweep adds one 0.5MB w slice.
Host side transposes/casts inputs (part of sharding) and transposes the
(C, L) bf16 per-core result back to the (B, L, C) fp32 output.
"""

import numpy as np
import ml_dtypes

import sys
for _p in ("/opt/trn_rl_repo", "/root/.axon_site/_ro/trn_rl_repo"):
    if _p not in sys.path:
        sys.path.append(_p)

B, T, C, K = 8, 4096, 512, 7
L = T - K + 1  # 4090
NCORES = 8
P = 128           # partitions
DC = C // P       # 4 channel chunks
NL = 512          # l-tile (one PSUM bank of fp32)
NLT = (L + NL - 1) // NL  # 8 l-tiles, last ragged (506)
QUAD = 4          # l-tiles per group
NQ = NLT // QUAD  # 2 groups
XBLK = 512        # x load chunk (columns)

_cache = {}


def _build():
    import concourse.bass as bass  # noqa: F401
    import concourse.mybir as mybir
    import concourse.tile as tile
    from concourse import bacc

    bf16 = mybir.dt.bfloat16
    f32 = mybir.dt.float32
    Tanh = mybir.ActivationFunctionType.Tanh

    nc = bacc.Bacc("TRN2", target_bir_lowering=False, debug=False,
                   num_devices=NCORES)

    xT_d = nc.dram_tensor("xT", [C, T], bf16, kind="ExternalInput")
    wT_d = nc.dram_tensor("wT", [K, C, C], bf16, kind="ExternalInput")
    outT_d = nc.dram_tensor("outT", [C, L], bf16, kind="ExternalOutput")

    with tile.TileContext(nc) as tc:
        with (
            tc.tile_pool(name="wpool", bufs=1) as wpool,
            tc.tile_pool(name="xpool", bufs=1) as xpool,
            tc.tile_pool(name="gpool", bufs=6) as gpool,
            tc.tile_pool(name="apool", bufs=8) as apool,
            tc.tile_pool(name="ppool", bufs=3) as ppool,
            tc.tile_pool(name="psum", bufs=8, space="PSUM") as psum_pool,
        ):
            # w_sb[cc][c_in, k, d] = weights[d, cc*128+c_in, k]
            w_sb = [wpool.tile([P, K, C], bf16, name=f"w_{cc}")
                    for cc in range(DC)]
            xT_sb = [xpool.tile([P, T], bf16, name=f"xt_{cc}")
                     for cc in range(DC)]

            def load_w(k):
                for cc in range(DC):
                    nc.sync.dma_start(w_sb[cc][:, k, :],
                                      wT_d.ap()[k, cc * P:(cc + 1) * P, :])

            def load_x(blk):
                c0 = blk * XBLK
                for cc in range(DC):
                    nc.sync.dma_start(
                        xT_sb[cc][:, c0:c0 + XBLK],
                        xT_d.ap()[cc * P:(cc + 1) * P, c0:c0 + XBLK])

            # DMA issue order == consumption order. (Finer splits were
            # tried and regress: more dma_starts serialize on the sync
            # dispatcher and smaller descriptors cut DMA efficiency.)
            for cc in range(DC):
                nc.sync.dma_start(w_sb[cc][:, 0, :],
                                  wT_d.ap()[0, cc * P:(cc + 1) * P, :])
                for blk in (0, 1):
                    c0 = blk * XBLK
                    nc.sync.dma_start(
                        xT_sb[cc][:, c0:c0 + XBLK],
                        xT_d.ap()[cc * P:(cc + 1) * P, c0:c0 + XBLK])
            for blk in range(2, 5):    # cols 0..2559 cover quad 0 (+halo)
                load_x(blk)
            for k in range(1, K):
                load_w(k)
            for blk in range(5, T // XBLK):
                load_x(blk)

            # PE clock ramps (~0.9 -> 2.4 GHz) over the first ~10us of
            # activity, and re-throttles after any >1us idle gap. Run
            # dummy matmuls on a scratch tile sized so the PE exits
            # warm-up right when the first k-sweep's ~3MB of operands
            # has landed (~18us) and can stream gap-free at full clock.
            # (The psum result is never read.)
            NWARM = 33
            warm = wpool.tile([P, NL], bf16, name="warm")
            nc.gpsimd.memset(warm[:], 1.0)
            warm_ps = psum_pool.tile([P, NL], f32, tag="ps", name="warm_ps")
            for i in range(NWARM):
                nc.tensor.matmul(warm_ps, warm[:, :P], warm,
                                 start=True, stop=(i == NWARM - 1))

            for lq in range(NQ):
                q0 = lq * QUAD * NL               # first output col
                qn = min(QUAD * NL, L - q0)       # 2048 / 2042
                acc = [None] * DC
                if lq == 0:
                    # Split the first k-sweep into two half-quad passes:
                    # the first pass only needs x cols 0..1535 (~2MB with
                    # w[k=0]), so matmuls stream gap-free while the rest
                    # of the quad's operands are still in flight.
                    g0 = [gpool.tile([P, QUAD * NL], bf16, tag="g",
                                     name=f"g0_{dc}") for dc in range(DC)]
                    for half in range(2):
                        for dc in range(DC):
                            ps = [psum_pool.tile([P, NL], f32, tag="ps",
                                                 name=f"ps0_{half}_{dc}_{i}")
                                  for i in range(2)]
                            for cc in range(DC):
                                lhsT = w_sb[cc][:, 0, dc * P:(dc + 1) * P]
                                for i in range(2):
                                    lt = half * 2 + i
                                    l0 = lt * NL
                                    nc.tensor.matmul(
                                        ps[i],
                                        lhsT,
                                        xT_sb[cc][:, l0 + K - 1:
                                                  l0 + K - 1 + NL],
                                        start=(cc == 0),
                                        stop=(cc == DC - 1),
                                    )
                            for i in range(2):
                                lt = half * 2 + i
                                nc.scalar.activation(
                                    g0[dc][:, lt * NL:(lt + 1) * NL],
                                    ps[i], Tanh)
                    for dc in range(DC):
                        a0 = apool.tile([P, QUAD * NL], bf16, tag="acc",
                                        name=f"acc0_{dc}")
                        nc.vector.tensor_mul(a0[:, :qn], g0[dc][:, :qn],
                                             xT_sb[dc][:, q0:q0 + qn])
                        acc[dc] = a0
                for k in range(K):
                    if lq == 0 and k == 0:
                        continue
                    for dc in range(DC):
                        ps = [psum_pool.tile([P, NL], f32, tag="ps",
                                             name=f"ps_{lq}_{k}_{dc}_{i}")
                              for i in range(QUAD)]
                        for cc in range(DC):
                            lhsT = w_sb[cc][:, k, dc * P:(dc + 1) * P]
                            for i in range(QUAD):
                                l0 = q0 + i * NL
                                nl = min(NL, L - l0)
                                nc.tensor.matmul(
                                    ps[i][:, :nl],
                                    lhsT,
                                    xT_sb[cc][:, l0 + K - 1: l0 + K - 1 + nl],
                                    start=(cc == 0),
                                    stop=(cc == DC - 1),
                                )
                        g = gpool.tile([P, QUAD * NL], bf16, tag="g",
                                       name=f"g_{lq}_{k}_{dc}")
                        last_unit = (lq == NQ - 1 and k == K - 1
                                     and dc == DC - 1)
                        if last_unit:
                            # Pipeline the final unit per l-tile so the
                            # epilogue after the last matmul is short.
                            nxt = apool.tile([P, QUAD * NL], bf16, tag="acc",
                                             name=f"accn_{lq}_{k}_{dc}")
                            for i in range(QUAD):
                                l0 = q0 + i * NL
                                nl = min(NL, L - l0)
                                o = i * NL
                                gs = g[:, o:o + nl]
                                nc.scalar.activation(gs, ps[i][:, :nl], Tanh)
                                prod = ppool.tile([P, QUAD * NL], bf16,
                                                  tag="prod",
                                                  name=f"prodL_{i}")
                                nc.vector.tensor_mul(
                                    prod[:, o:o + nl], gs,
                                    xT_sb[dc][:, l0 + k:l0 + k + nl])
                                nc.vector.tensor_add(
                                    nxt[:, o:o + nl],
                                    acc[dc][:, o:o + nl], prod[:, o:o + nl])
                                nc.sync.dma_start(
                                    outT_d.ap()[dc * P:(dc + 1) * P,
                                                l0:l0 + nl],
                                    nxt[:, o:o + nl])
                            acc[dc] = None
                            continue
                        for i in range(QUAD):
                            l0 = q0 + i * NL
                            nl = min(NL, L - l0)
                            nc.scalar.activation(
                                g[:, i * NL:i * NL + nl], ps[i][:, :nl], Tanh)
                        xu = xT_sb[dc][:, q0 + k:q0 + k + qn]
                        if acc[dc] is None:
                            a0 = apool.tile([P, QUAD * NL], bf16, tag="acc",
                                            name=f"acc_{lq}_{k}_{dc}")
                            nc.vector.tensor_mul(a0[:, :qn], g[:, :qn], xu)
                            acc[dc] = a0
                        else:
                            prod = ppool.tile([P, QUAD * NL], bf16,
                                              tag="prod",
                                              name=f"prod_{lq}_{k}_{dc}")
                            nc.vector.tensor_mul(prod[:, :qn], g[:, :qn], xu)
                            nxt = apool.tile([P, QUAD * NL], bf16, tag="acc",
                                             name=f"accn_{lq}_{k}_{dc}")
                            nc.vector.tensor_add(nxt[:, :qn], acc[dc][:, :qn],
                                                 prod[:, :qn])
                            acc[dc] = nxt
                for dc in range(DC):
                    if acc[dc] is not None:
                        nc.sync.dma_start(
                            outT_d.ap()[dc * P:(dc + 1) * P, q0:q0 + qn],
                            acc[dc][:, :qn])

    nc.compile()
    return nc


def _prep_inputs(x, weights):
    bf = ml_dtypes.bfloat16
    wT = np.transpose(weights, (2, 1, 0)).astype(bf)  # (K, C, D)
    wT = np.ascontiguousarray(wT)
    in_maps = []
    for b in range(B):
        xT = x[b].T.astype(bf)  # (C, T) contiguous
        in_maps.append({"xT": xT, "wT": wT})
    return in_maps


def kernel(x, weights):
    x = np.asarray(x, dtype=np.float32)
    weights = np.asarray(weights, dtype=np.float32)
    assert x.shape == (B, T, C) and weights.shape == (C, C, K)

    from concourse.bass_utils import run_bass_kernel_spmd

    if "nc" not in _cache:
        _cache["nc"] = _build()
    nc = _cache["nc"]

    in_maps = _prep_inputs(x, weights)
    res = run_bass_kernel_spmd(nc, in_maps, list(range(NCORES)))

    out = np.empty((B, L, C), dtype=np.float32)
    for b in range(B):
        out[b] = res.results[b]["outT"].astype(np.float32).T
    return out


if __name__ == "__main__":
    rng = np.random.default_rng(0)
    x = rng.standard_normal((B, T, C), dtype=np.float32)
    w = (rng.standard_normal((C, C, K), dtype=np.float32)
         / np.sqrt(np.float32(C * K)))
    out = kernel(x, w)
    print("out", out.shape, out.dtype, float(np.abs(out).max()))
